# revision 35
# baseline (speedup 1.0000x reference)
"""Trainium2 Bass kernel for cosine-similarity ("sparse") attention.

Reference computation (B=2, C=512, N=2048, H=16, D=64, SCALE=8):
    qkv = Wqkv @ x                          # 1x1 conv
    q,k,v -> [B,H,D,N]
    q = l2norm(q, over D) * q_scale ; k = l2norm(k, over D) * k_scale
    sim = (q^T k) * 8 ; attn = softmax(sim, over keys)
    out = Wout @ (attn @ v) + bout

Sharding: 32 (batch, head) pairs across 8 cores -> each core owns one batch
(b = core//4) and 4 heads (h0 = 4*(core%4)).  Each core projects q/k/v for
its heads, runs attention, and computes a partial output projection
Wout[:, its-heads] @ y + bout/4.  Host sums the 4 partials per batch.

Device-side schedule (per core), tuned against NTFF traces:
  - Phase 2 is ACT(exp)-bound at ~1.1us per j-step (128 steps of
    [128,1024] Exp).  Everything else is arranged to hide inside that:
    the av pair runs one j-step behind the sim pair ACROSS block
    boundaries; out-proj column chunks are staggered one per 8 j-steps;
    the o-accumulators are evacuated PSUM->SBUF immediately so their
    banks recycle without waiting on the softmax-denominator divide.
  - qn/kn/y/Wout are bf16 (halves sim LDWEIGHTS + out-proj cost;
    ~0.5-1% rms error, well under the 2e-2 gate).  Norms/sumsq/exp
    stay fp32.
  - Softmax max-subtraction is skipped: sim = 8*cosine is in [-8, 8].
  - l2norm: ones-indicator matmul for sumsq, fused PSUM->SBUF Sqrt on
    ACT, DVE reciprocal; the m=0 chain is emitted before the m=1
    projections so attention can start right after the v projection,
    and the m=1 chain overlaps the first attention block.
  - Startup: small consts + x ride sync/scalar/gpsimd trigger queues
    first; wo comes after the x triggers; the softmax ones-column is a
    GpSimd memset (a scatter DMA here would flood the queues that are
    busy fetching the instruction stream at kernel start).
"""

import os
import sys

import numpy as np

sys.path.insert(0, "/opt/trn_rl_repo")

import concourse.bass as bass  # noqa: E402
import concourse.mybir as mybir  # noqa: E402
from concourse import bacc, tile  # noqa: E402
from concourse.bass_utils import run_bass_kernel_spmd  # noqa: E402

F32 = mybir.dt.float32
F32R = mybir.dt.float32r
BF16 = mybir.dt.bfloat16
AF = mybir.ActivationFunctionType
OP = mybir.AluOpType

B, C, N = 2, 512, 2048
HEADS, D = 16, 64
SCALE = 8.0
NCORES = 8
HPC = 4  # heads per core

_CACHED_NC = None
LAST_RESULTS = None
EXTRA_RUN_KWARGS = {}


def build_nc():
    nc = bacc.Bacc(None, target_bir_lowering=False)

    x_d = nc.declare_dram_parameter("x", [C, N], F32R, isOutput=False)
    wqT_d = nc.declare_dram_parameter("wqT", [C, HPC * D], F32R, isOutput=False)
    wkT_d = nc.declare_dram_parameter("wkT", [C, HPC * D], F32R, isOutput=False)
    wvT_d = nc.declare_dram_parameter("wvT", [C, HPC * D], F32R, isOutput=False)
    woT_d = nc.declare_dram_parameter("woT", [HPC * D, C], BF16, isOutput=False)
    qsks8_d = nc.declare_dram_parameter("qsks8", [128, 1], F32, isOutput=False)
    onesw_d = nc.declare_dram_parameter("onesw", [128, 33], F32R, isOutput=False)
    biasq_d = nc.declare_dram_parameter("biasq", [C, 1], F32, isOutput=False)
    out_d = nc.declare_dram_parameter("out", [C, N], F32, isOutput=True)

    NQT = N // 512  # 4 query chunks of 512
    NJ = N // 128  # 16 key chunks of 128
    NCT = C // 128  # 4 channel chunks of 128

    with tile.TileContext(nc) as tc:
        with (
            tc.tile_pool(name="const", bufs=1) as const,
            tc.tile_pool(name="persist", bufs=1) as persist,
            tc.tile_pool(name="dramp", bufs=1, space="DRAM") as dramp,
        ):
            qsks8_sb = const.tile([128, 1], F32, name="qsks8", tag="qsks8")
            nc.sync.dma_start(qsks8_sb[:], qsks8_d[:])
            biasq_sb = const.tile([128, NCT], F32, name="biasq", tag="biasq")
            nc.sync.dma_start(
                biasq_sb[:], biasq_d[:].rearrange("(c p) o -> p (c o)", p=128)
            )
            # indicator weights: col 0 sums partitions 0-63 (head A), col 32
            # sums partitions 64-127 (head B); middle cols write zeros so the
            # [33, 512] sumsq psum rows land 32-aligned.  (DMA'd from host:
            # engines cannot memset float32r tiles.)
            ones_w = const.tile([128, 33], F32R, name="ones_w", tag="ones_w")
            nc.sync.dma_start(ones_w[:], onesw_d[:])
            wo_sb = [
                const.tile([128, C], BF16, name=f"wo{m}", tag=f"wo{m}")
                for m in range(2)
            ]

            # persistent tensors
            qn = [persist.tile([128, N], BF16, name=f"qn{m}", tag=f"qn{m}") for m in range(2)]
            kn = [persist.tile([128, N], BF16, name=f"kn{m}", tag=f"kn{m}") for m in range(2)]
            y = [
                [
                    persist.tile([128, 512], BF16, name=f"y{m}_{qt}", tag=f"y{m}_{qt}")
                    for qt in range(4)
                ]
                for m in range(2)
            ]
            vext = persist.tile([128, NJ, HPC, D + 1], BF16, name="vext", tag="vext")
            inv_dram = dramp.tile([8, N], F32, name="inv_dram", tag="inv_dram")
            # softmax-denominator ones column: engine memset (bf16) instead
            # of a 2-byte-granular scatter DMA
            nc.gpsimd.memset(vext[:, :, :, D : D + 1], 1.0)

            # ---------------- phase 1: projections + norms ----------------
            with (
                tc.tile_pool(name="xw", bufs=1) as xw,
                tc.tile_pool(name="raw", bufs=1) as rawp,
                tc.tile_pool(name="sq", bufs=3) as sqp,
                tc.tile_pool(name="bb", bufs=4) as bbp,
                tc.tile_pool(name="prps", bufs=4, space="PSUM") as prps,
                tc.tile_pool(name="ssps", bufs=2, space="PSUM") as ssps,
            ):
                # sqrt(sumsq) tiles; head A at row 0 / head B at row 32,
                # filled chunk-by-chunk by fused PSUM->SBUF Sqrt on ACT,
                # then inverted whole by one DVE reciprocal per (t, m)
                srt_tm = [
                    [
                        rawp.tile([33, N], F32, name=f"srt{t}{m}", tag=f"srt{t}{m}")
                        for m in range(2)
                    ]
                    for t in range(2)
                ]
                inv_tm = [
                    [
                        rawp.tile([33, N], F32, name=f"inv{t}{m}", tag=f"inv{t}{m}")
                        for m in range(2)
                    ]
                    for t in range(2)
                ]
                # DMA order tuned for earliest first matmul: wq, x[nt0], wk,
                # x[nt1..3], wv, wo; triggers spread over the trigger queues
                wq_all = xw.tile([128, NCT, HPC * D], F32R, name="wq_all", tag="wq_all")
                nc.scalar.dma_start(
                    wq_all[:], wqT_d[:].rearrange("(c p) d -> p c d", p=128)
                )
                wq_sb = [wq_all[:, c, :] for c in range(NCT)]
                dma_engs = [nc.sync, nc.scalar, nc.gpsimd, nc.sync]
                x_sb = [[None] * NQT for _ in range(NCT)]
                for c in range(NCT):
                    t = xw.tile([128, 512], F32R, name=f"x{c}_0", tag=f"x{c}_0")
                    dma_engs[c].dma_start(t[:], x_d[c * 128 : (c + 1) * 128, 0:512])
                    x_sb[c][0] = t
                wk_all = xw.tile([128, NCT, HPC * D], F32R, name="wk_all", tag="wk_all")
                nc.gpsimd.dma_start(
                    wk_all[:], wkT_d[:].rearrange("(c p) d -> p c d", p=128)
                )
                wk_sb = [wk_all[:, c, :] for c in range(NCT)]
                wv_all = xw.tile([128, NCT, HPC * D], F32R, name="wv_all", tag="wv_all")
                nc.scalar.dma_start(
                    wv_all[:], wvT_d[:].rearrange("(c p) d -> p c d", p=128)
                )
                wv_sb = [wv_all[:, c, :] for c in range(NCT)]
                for nt in range(1, NQT):
                    for c in range(NCT):
                        t = xw.tile([128, 512], F32R, name=f"x{c}_{nt}", tag=f"x{c}_{nt}")
                        dma_engs[c].dma_start(
                            t[:], x_d[c * 128 : (c + 1) * 128, nt * 512 : (nt + 1) * 512]
                        )
                        x_sb[c][nt] = t
                for m in range(2):
                    nc.gpsimd.dma_start(
                        wo_sb[m][:], woT_d[m * 128 : (m + 1) * 128, :]
                    )

                # sumsq matmuls are emitted one proj-chunk late so the PE
                # never waits on the ACT square of the current chunk
                pend_ss = []

                def emit_ss(limit):
                    while len(pend_ss) > limit:
                        sq_t, ti_, m_, nt_ = pend_ss.pop(0)
                        ss = ssps.tile([33, 512], F32, name="ss", tag="ss")
                        nc.tensor.matmul(
                            ss[:], lhsT=(ones_w[:]), rhs=(sq_t[:]), start=True, stop=True
                        )
                        nc.scalar.activation(
                            srt_tm[ti_][m_][:, nt_ * 512 : (nt_ + 1) * 512],
                            ss[:],
                            AF.Sqrt,
                        )

                def proj_chunk(m, w_sb, raws, ti, nt):
                    ps = prps.tile([128, 512], F32, name="pr", tag="pr")
                    for c in range(NCT):
                        nc.tensor.matmul(
                            ps[:],
                            lhsT=(w_sb[c][:, m * 128 : (m + 1) * 128]),
                            rhs=(x_sb[c][nt][:]),
                            start=(c == 0),
                            stop=(c == NCT - 1),
                        )
                    emit_ss(1)
                    nc.vector.tensor_copy(
                        raws[m][:, nt * 512 : (nt + 1) * 512], ps[:]
                    )
                    sq = sqp.tile([128, 512], F32R, name="sq", tag="sq")
                    nc.scalar.activation(sq[:], ps[:], AF.Square)
                    pend_ss.append((sq, ti, m, nt))

                def norm_head(m):
                    # reciprocals + the inverse-norm row DMA roundtrip with
                    # the 64-partition broadcast.  No DVE op here waits on a
                    # DMA, so the DVE queue never blocks behind this.
                    bts = []
                    for ti in range(2):
                        nc.vector.reciprocal_approx_fast(
                            inv_tm[ti][m][:], srt_tm[ti][m][:]
                        )
                        nc.sync.dma_start(
                            inv_dram[4 * ti + 2 * m : 4 * ti + 2 * m + 2, :],
                            inv_tm[ti][m][0:33:32, :],
                        )
                    bt_engs = [nc.sync, nc.gpsimd]
                    for ti in range(2):
                        rowA = 4 * ti + 2 * m
                        bt = bbp.tile([128, N], F32, name="bt", tag="bt")
                        eng = bt_engs[ti]
                        eng.dma_start(
                            bt[0:64, :].unsqueeze(1),
                            inv_dram[rowA : rowA + 1, :].partition_broadcast(64),
                        )
                        eng.dma_start(
                            bt[64:128, :].unsqueeze(1),
                            inv_dram[rowA + 1 : rowA + 2, :].partition_broadcast(64),
                        )
                        bts.append(bt)
                    return bts

                def norm_tail(m, bts):
                    # the actual qn/kn scaling; emitted a projection batch
                    # later so the broadcast DMA has landed by the time the
                    # DVE/GpSimd queues reach these
                    nc.vector.scalar_tensor_tensor(
                        qn[m][:], qn[m][:], qsks8_sb[:], bts[0][:],
                        OP.mult, OP.mult,
                    )
                    nc.gpsimd.tensor_tensor(
                        kn[m][:], kn[m][:], bts[1][:], OP.mult
                    )

                # m=0 projections, then its norm chain split around the m=1
                # projections (the DMA roundtrip overlaps them); m=1's chain
                # splits around vproj and overlaps the first attention block
                for nt in range(NQT):
                    proj_chunk(0, wq_sb, qn, 0, nt)
                    proj_chunk(0, wk_sb, kn, 1, nt)
                emit_ss(0)
                bts0 = norm_head(0)
                for nt in range(NQT):
                    proj_chunk(1, wq_sb, qn, 0, nt)
                    proj_chunk(1, wk_sb, kn, 1, nt)
                emit_ss(0)
                norm_tail(0, bts0)

                # v projection; vext copies ride the otherwise-idle ACT
                # (same activation table as Square/Sqrt) so the psv slots
                # recycle without touching the DVE queue
                for nm_ in range(NJ):
                    psv = prps.tile([128, HPC * D], F32, name="prv", tag="pr")
                    for c in range(NCT):
                        nc.tensor.matmul(
                            psv[:],
                            lhsT=(
                                x_sb[c][nm_ // 4][
                                    :, (nm_ % 4) * 128 : (nm_ % 4) * 128 + 128
                                ]
                            ),
                            rhs=(wv_sb[c][:]),
                            start=(c == 0),
                            stop=(c == NCT - 1),
                        )
                    nc.scalar.activation(
                        vext[:, nm_, :, 0:D],
                        psv[:].rearrange("p (h d) -> p h d", h=HPC),
                        AF.Copy,
                    )
                bts1 = norm_head(1)
                norm_tail(1, bts1)

            # ---------------- phase 2: attention + fused out-proj ----------
            with (
                tc.tile_pool(name="simps", bufs=2, space="PSUM") as simps,
                tc.tile_pool(name="ops", bufs=3, space="PSUM") as ops,
                tc.tile_pool(name="ppps", bufs=1, space="PSUM") as ppps,
                tc.tile_pool(name="at", bufs=4) as atp,
                tc.tile_pool(name="nrm", bufs=4) as nrm,
                tc.tile_pool(name="fin", bufs=4) as finp,
            ):
                out_engs = [nc.sync, nc.gpsimd, nc.sync, nc.gpsimd]

                def out_proj_ct(qt, ct):
                    qs_ = slice(qt * 512, (qt + 1) * 512)
                    pp = ppps.tile([128, 512], F32, name="pp", tag="pp")
                    for m in range(2):
                        nc.tensor.matmul(
                            pp[:],
                            lhsT=(wo_sb[m][:, ct * 128 : (ct + 1) * 128]),
                            rhs=(y[m][qt][:]),
                            start=(m == 0),
                            stop=(m == 1),
                        )
                    ot = finp.tile([128, 512], F32, name="ot", tag="ot")
                    nc.vector.tensor_scalar_add(
                        ot[:], pp[:], biasq_sb[:, ct : ct + 1]
                    )
                    out_engs[ct].dma_start(
                        out_d[ct * 128 : (ct + 1) * 128, qs_], ot[:]
                    )

                def av_pair(at_t, j_, oA, oB, hA, hB):
                    nc.tensor.matmul(
                        oA[:],
                        lhsT=(vext[:, j_, hA, :]),
                        rhs=(at_t[:, 0:512]),
                        start=(j_ == 0),
                        stop=(j_ == NJ - 1),
                    )
                    nc.tensor.matmul(
                        oB[:],
                        lhsT=(vext[:, j_, hB, :]),
                        rhs=(at_t[:, 512:1024]),
                        start=(j_ == 0),
                        stop=(j_ == NJ - 1),
                    )

                def normalize(oA, oB, m, qt):
                    # evacuate the accumulators PSUM->SBUF first: the PSUM
                    # banks recycle into the next block without waiting on
                    # the softmax divide chain.  The denominator row gets its
                    # own partition-0 tile (partition_broadcast broadcasts
                    # partition 0).
                    parts = []
                    for o_ps in (oA, oB):
                        oc = nrm.tile([D, 512], F32, name="oc", tag="oc")
                        nc.vector.tensor_copy(oc[:], o_ps[0:D, :])
                        rsb = nrm.tile([1, 512], F32, name="rsb", tag="rsb")
                        nc.vector.tensor_copy(rsb[:], o_ps[D : D + 1, :])
                        parts.append((oc, rsb))
                    for (oc, rsb), base in ((parts[0], 0), (parts[1], 64)):
                        br = nrm.tile([64, 512], F32, name="br", tag="br")
                        nc.gpsimd.partition_broadcast(br[:], rsb[:], channels=64)
                        bri = nrm.tile([64, 512], F32, name="bri", tag="bri")
                        nc.vector.reciprocal_approx_fast(bri[:], br[:])
                        nc.vector.tensor_tensor(
                            y[m][qt][base : base + 64, :],
                            oc[:],
                            bri[:],
                            OP.mult,
                        )

                # the av pair runs one j-step behind the sim pair, ACROSS
                # (m, qt) block boundaries, so the PE's in-order queue never
                # blocks on the current j's exp and blocks hand off
                # seamlessly.  Out-proj column chunks for a finished qt are
                # staggered one per 8 j-steps into the following blocks.
                pend_av = [None]
                pend_pp = []

                def flush_pend():
                    if pend_av[0] is None:
                        return
                    at_t, j_, oA, oB, m_, qt_ = pend_av[0]
                    pend_av[0] = None
                    av_pair(at_t, j_, oA, oB, 2 * m_, 2 * m_ + 1)
                    if j_ == NJ - 1:
                        normalize(oA, oB, m_, qt_)
                        if m_ == 1:
                            pend_pp.extend((qt_, ct) for ct in range(NCT))

                for qt in range(NQT):
                    for m in range(2):
                        qs_ = slice(qt * 512, (qt + 1) * 512)
                        oA = ops.tile([D + 1, 512], F32, name="oA", tag="o")
                        oB = ops.tile([D + 1, 512], F32, name="oB", tag="o")
                        for j in range(NJ):
                            js = slice(j * 128, (j + 1) * 128)
                            sim = simps.tile([128, 1024], F32, name="sim", tag="sim")
                            nc.tensor.matmul(
                                sim[:, 0:512],
                                lhsT=(kn[m][0:64, js]),
                                rhs=(qn[m][0:64, qs_]),
                                start=True,
                                stop=True,
                            )
                            nc.tensor.matmul(
                                sim[:, 512:1024],
                                lhsT=(kn[m][64:128, js]),
                                rhs=(qn[m][64:128, qs_]),
                                start=True,
                                stop=True,
                            )
                            flush_pend()
                            if pend_pp and j % 8 == 2:
                                out_proj_ct(*pend_pp.pop(0))
                            at = atp.tile([128, 1024], BF16, name="at", tag="at")
                            nc.scalar.activation(at[:], sim[:], AF.Exp)
                            pend_av[0] = (at, j, oA, oB, m, qt)
                        # fall through: av(15) + normalize emitted after the
                        # next block's first sim pair
                flush_pend()
                while pend_pp:
                    out_proj_ct(*pend_pp.pop(0))

    nc.finalize()
    return nc


def kernel(x, Wqkv, q_scale, k_scale, Wout, bout):
    global _CACHED_NC, LAST_RESULTS
    x = np.asarray(x, dtype=np.float32)
    Wqkv = np.asarray(Wqkv, dtype=np.float32)
    q_scale = np.asarray(q_scale, dtype=np.float32)
    k_scale = np.asarray(k_scale, dtype=np.float32)
    Wout = np.asarray(Wout, dtype=np.float32)
    bout = np.asarray(bout, dtype=np.float32)

    if _CACHED_NC is None:
        _CACHED_NC = build_nc()
    nc = _CACHED_NC

    H_DIM = HEADS * D
    qsks8 = np.tile((SCALE * q_scale * k_scale).astype(np.float32), 2)[:, None]
    qsks8 = np.ascontiguousarray(qsks8)
    biasq = np.ascontiguousarray((bout / 4.0).astype(np.float32)[:, None])
    onesw = np.zeros((128, 33), dtype=np.float32)
    onesw[0:64, 0] = 1.0
    onesw[64:128, 32] = 1.0
    import ml_dtypes

    in_maps = []
    for core in range(NCORES):
        b = core // 4
        h0 = HPC * (core % 4)
        rs = slice(h0 * D, h0 * D + HPC * D)
        wq = Wqkv[0:H_DIM][rs]
        wk = Wqkv[H_DIM : 2 * H_DIM][rs]
        wv = Wqkv[2 * H_DIM : 3 * H_DIM][rs]
        in_maps.append(
            {
                "x": np.ascontiguousarray(x[b]),
                "wqT": np.ascontiguousarray(wq.T),
                "wkT": np.ascontiguousarray(wk.T),
                "wvT": np.ascontiguousarray(wv.T),
                "woT": np.ascontiguousarray(Wout[:, rs].T).astype(ml_dtypes.bfloat16),
                "qsks8": qsks8,
                "onesw": onesw,
                "biasq": biasq,
            }
        )

    res = run_bass_kernel_spmd(
        nc,
        in_maps,
        core_ids=list(range(NCORES)),
        trace=bool(os.environ.get("BASS_TRACE")),
        **EXTRA_RUN_KWARGS,
    )
    LAST_RESULTS = res

    outs = [np.asarray(res.results[i]["out"], dtype=np.float32) for i in range(NCORES)]
    full = np.empty((B, C, N), dtype=np.float32)
    full[0] = outs[0] + outs[1] + outs[2] + outs[3]
    full[1] = outs[4] + outs[5] + outs[6] + outs[7]
    return full


# revision 37
# speedup vs baseline: 1.1961x; 1.1961x over previous
"""Trainium2 Bass kernel for cosine-similarity ("sparse") attention.

Reference computation (B=2, C=512, N=2048, H=16, D=64, SCALE=8):
    qkv = Wqkv @ x                          # 1x1 conv
    q,k,v -> [B,H,D,N]
    q = l2norm(q, over D) * q_scale ; k = l2norm(k, over D) * k_scale
    sim = (q^T k) * 8 ; attn = softmax(sim, over keys)
    out = Wout @ (attn @ v) + bout

Sharding: 32 (batch, head) pairs across 8 cores -> each core owns one batch
(b = core//4) and 4 heads (h0 = 4*(core%4)).  Each core projects q/k/v for
its heads, runs attention, and computes a partial output projection
Wout[:, its-heads] @ y + bout/4.  Host sums the 4 partials per batch.

Device-side schedule (per core), tuned against NTFF traces:
  - Phase 2 is ACT(exp)-bound at ~1.1us per j-step (128 steps of
    [128,1024] Exp).  Everything else is arranged to hide inside that:
    the av pair runs one j-step behind the sim pair ACROSS block
    boundaries; out-proj column chunks are staggered one per 8 j-steps;
    the o-accumulators are evacuated PSUM->SBUF immediately so their
    banks recycle without waiting on the softmax-denominator divide.
  - qn/kn/y/Wout are bf16 (halves sim LDWEIGHTS + out-proj cost;
    ~0.5-1% rms error, well under the 2e-2 gate).  Norms/sumsq/exp
    stay fp32.
  - Softmax max-subtraction is skipped: sim = 8*cosine is in [-8, 8].
  - l2norm: ones-indicator matmul for sumsq, fused PSUM->SBUF Sqrt on
    ACT, DVE reciprocal; the m=0 chain is emitted before the m=1
    projections so attention can start right after the v projection,
    and the m=1 chain overlaps the first attention block.
  - Startup: small consts + x ride sync/scalar/gpsimd trigger queues
    first; wo comes after the x triggers; the softmax ones-column is a
    GpSimd memset (a scatter DMA here would flood the queues that are
    busy fetching the instruction stream at kernel start).
"""

import os
import sys

import numpy as np

sys.path.insert(0, "/opt/trn_rl_repo")

import concourse.bass as bass  # noqa: E402
import concourse.mybir as mybir  # noqa: E402
from concourse import bacc, tile  # noqa: E402
from concourse.bass_utils import run_bass_kernel_spmd  # noqa: E402

F32 = mybir.dt.float32
F32R = mybir.dt.float32r
BF16 = mybir.dt.bfloat16
AF = mybir.ActivationFunctionType
OP = mybir.AluOpType

B, C, N = 2, 512, 2048
HEADS, D = 16, 64
SCALE = 8.0
NCORES = 8
HPC = 4  # heads per core

_CACHED_NC = None
LAST_RESULTS = None
EXTRA_RUN_KWARGS = {}


def build_nc():
    nc = bacc.Bacc(None, target_bir_lowering=False)

    x_d = nc.declare_dram_parameter("x", [C, N], F32R, isOutput=False)
    wqT_d = nc.declare_dram_parameter("wqT", [C, HPC * D], F32R, isOutput=False)
    wkT_d = nc.declare_dram_parameter("wkT", [C, HPC * D], F32R, isOutput=False)
    wvT_d = nc.declare_dram_parameter("wvT", [C, HPC * D], F32R, isOutput=False)
    woT_d = nc.declare_dram_parameter("woT", [HPC * D, C], BF16, isOutput=False)
    qsks8_d = nc.declare_dram_parameter("qsks8", [128, 1], F32, isOutput=False)
    onesw_d = nc.declare_dram_parameter("onesw", [128, 33], F32R, isOutput=False)
    biasq_d = nc.declare_dram_parameter("biasq", [C, 1], F32, isOutput=False)
    out_d = nc.declare_dram_parameter("out", [C, N], F32, isOutput=True)

    NQT = N // 512  # 4 query chunks of 512
    NJ = N // 128  # 16 key chunks of 128
    NCT = C // 128  # 4 channel chunks of 128

    with tile.TileContext(nc) as tc:
        with (
            tc.tile_pool(name="const", bufs=1) as const,
            tc.tile_pool(name="persist", bufs=1) as persist,
            tc.tile_pool(name="dramp", bufs=1, space="DRAM") as dramp,
        ):
            qsks8_sb = const.tile([128, 1], F32, name="qsks8", tag="qsks8")
            nc.sync.dma_start(qsks8_sb[:], qsks8_d[:])
            biasq_sb = const.tile([128, NCT], F32, name="biasq", tag="biasq")
            nc.sync.dma_start(
                biasq_sb[:], biasq_d[:].rearrange("(c p) o -> p (c o)", p=128)
            )
            # indicator weights: col 0 sums partitions 0-63 (head A), col 32
            # sums partitions 64-127 (head B); middle cols write zeros so the
            # [33, 512] sumsq psum rows land 32-aligned.  (DMA'd from host:
            # engines cannot memset float32r tiles.)
            ones_w = const.tile([128, 33], F32R, name="ones_w", tag="ones_w")
            nc.sync.dma_start(ones_w[:], onesw_d[:])
            wo_sb = [
                const.tile([128, C], BF16, name=f"wo{m}", tag=f"wo{m}")
                for m in range(2)
            ]

            # persistent tensors
            qn = [persist.tile([128, N], BF16, name=f"qn{m}", tag=f"qn{m}") for m in range(2)]
            kn = [persist.tile([128, N], BF16, name=f"kn{m}", tag=f"kn{m}") for m in range(2)]
            y = [
                [
                    persist.tile([128, 512], BF16, name=f"y{m}_{qt}", tag=f"y{m}_{qt}")
                    for qt in range(4)
                ]
                for m in range(2)
            ]
            vext = persist.tile([128, NJ, HPC, D + 1], BF16, name="vext", tag="vext")
            inv_dram = dramp.tile([8, N], F32, name="inv_dram", tag="inv_dram")
            # softmax-denominator ones column: engine memset (bf16) instead
            # of a 2-byte-granular scatter DMA
            nc.gpsimd.memset(vext[:, :, :, D : D + 1], 1.0)

            # ---------------- phase 1: projections + norms ----------------
            with (
                tc.tile_pool(name="xw", bufs=1) as xw,
                tc.tile_pool(name="raw", bufs=1) as rawp,
                tc.tile_pool(name="sq", bufs=3) as sqp,
                tc.tile_pool(name="bb", bufs=4) as bbp,
                tc.tile_pool(name="prps", bufs=4, space="PSUM") as prps,
                tc.tile_pool(name="ssps", bufs=2, space="PSUM") as ssps,
            ):
                # sqrt(sumsq) tiles; head A at row 0 / head B at row 32,
                # filled chunk-by-chunk by fused PSUM->SBUF Sqrt on ACT,
                # then inverted whole by one DVE reciprocal per (t, m)
                srt_tm = [
                    [
                        rawp.tile([33, N], F32, name=f"srt{t}{m}", tag=f"srt{t}{m}")
                        for m in range(2)
                    ]
                    for t in range(2)
                ]
                inv_tm = [
                    [
                        rawp.tile([33, N], F32, name=f"inv{t}{m}", tag=f"inv{t}{m}")
                        for m in range(2)
                    ]
                    for t in range(2)
                ]
                # DMA order tuned for earliest first matmul: wq, x[nt0], wk,
                # x[nt1..3], wv, wo; triggers spread over the trigger queues
                wq_all = xw.tile([128, NCT, HPC * D], F32R, name="wq_all", tag="wq_all")
                nc.scalar.dma_start(
                    wq_all[:], wqT_d[:].rearrange("(c p) d -> p c d", p=128)
                )
                wq_sb = [wq_all[:, c, :] for c in range(NCT)]
                dma_engs = [nc.sync, nc.scalar, nc.gpsimd, nc.sync]
                x_sb = [[None] * NQT for _ in range(NCT)]
                for c in range(NCT):
                    t = xw.tile([128, 512], F32R, name=f"x{c}_0", tag=f"x{c}_0")
                    dma_engs[c].dma_start(t[:], x_d[c * 128 : (c + 1) * 128, 0:512])
                    x_sb[c][0] = t
                wk_all = xw.tile([128, NCT, HPC * D], F32R, name="wk_all", tag="wk_all")
                nc.gpsimd.dma_start(
                    wk_all[:], wkT_d[:].rearrange("(c p) d -> p c d", p=128)
                )
                wk_sb = [wk_all[:, c, :] for c in range(NCT)]
                wv_all = xw.tile([128, NCT, HPC * D], F32R, name="wv_all", tag="wv_all")
                nc.scalar.dma_start(
                    wv_all[:], wvT_d[:].rearrange("(c p) d -> p c d", p=128)
                )
                wv_sb = [wv_all[:, c, :] for c in range(NCT)]
                for nt in range(1, NQT):
                    for c in range(NCT):
                        t = xw.tile([128, 512], F32R, name=f"x{c}_{nt}", tag=f"x{c}_{nt}")
                        dma_engs[c].dma_start(
                            t[:], x_d[c * 128 : (c + 1) * 128, nt * 512 : (nt + 1) * 512]
                        )
                        x_sb[c][nt] = t
                for m in range(2):
                    nc.gpsimd.dma_start(
                        wo_sb[m][:], woT_d[m * 128 : (m + 1) * 128, :]
                    )

                # sumsq matmuls are emitted one proj-chunk late so the PE
                # never waits on the ACT square of the current chunk
                pend_ss = []

                def emit_ss(limit):
                    while len(pend_ss) > limit:
                        sq_t, ti_, m_, nt_ = pend_ss.pop(0)
                        ss = ssps.tile([33, 512], F32, name="ss", tag="ss")
                        nc.tensor.matmul(
                            ss[:], lhsT=(ones_w[:]), rhs=(sq_t[:]), start=True, stop=True
                        )
                        nc.scalar.activation(
                            srt_tm[ti_][m_][:, nt_ * 512 : (nt_ + 1) * 512],
                            ss[:],
                            AF.Sqrt,
                        )

                def proj_chunk(m, w_sb, raws, ti, nt):
                    ps = prps.tile([128, 512], F32, name="pr", tag="pr")
                    for c in range(NCT):
                        nc.tensor.matmul(
                            ps[:],
                            lhsT=(w_sb[c][:, m * 128 : (m + 1) * 128]),
                            rhs=(x_sb[c][nt][:]),
                            start=(c == 0),
                            stop=(c == NCT - 1),
                        )
                    emit_ss(1)
                    nc.vector.tensor_copy(
                        raws[m][:, nt * 512 : (nt + 1) * 512], ps[:]
                    )
                    sq = sqp.tile([128, 512], F32R, name="sq", tag="sq")
                    nc.scalar.activation(sq[:], ps[:], AF.Square)
                    pend_ss.append((sq, ti, m, nt))

                def norm_head(m):
                    # reciprocals + the inverse-norm row DMA roundtrip with
                    # the 64-partition broadcast.  No DVE op here waits on a
                    # DMA, so the DVE queue never blocks behind this.
                    bts = []
                    for ti in range(2):
                        nc.vector.reciprocal_approx_fast(
                            inv_tm[ti][m][:], srt_tm[ti][m][:]
                        )
                        nc.sync.dma_start(
                            inv_dram[4 * ti + 2 * m : 4 * ti + 2 * m + 2, :],
                            inv_tm[ti][m][0:33:32, :],
                        )
                    bt_engs = [nc.sync, nc.gpsimd]
                    for ti in range(2):
                        rowA = 4 * ti + 2 * m
                        bt = bbp.tile([128, N], F32, name="bt", tag="bt")
                        eng = bt_engs[ti]
                        eng.dma_start(
                            bt[0:64, :].unsqueeze(1),
                            inv_dram[rowA : rowA + 1, :].partition_broadcast(64),
                        )
                        eng.dma_start(
                            bt[64:128, :].unsqueeze(1),
                            inv_dram[rowA + 1 : rowA + 2, :].partition_broadcast(64),
                        )
                        bts.append(bt)
                    return bts

                def norm_tail(m, bts):
                    # the actual qn/kn scaling; emitted a projection batch
                    # later so the broadcast DMA has landed by the time the
                    # DVE/GpSimd queues reach these
                    nc.vector.scalar_tensor_tensor(
                        qn[m][:], qn[m][:], qsks8_sb[:], bts[0][:],
                        OP.mult, OP.mult,
                    )
                    nc.gpsimd.tensor_tensor(
                        kn[m][:], kn[m][:], bts[1][:], OP.mult
                    )

                # m=0 projections, then its norm chain split around the m=1
                # projections (the DMA roundtrip overlaps them); m=1's chain
                # splits around vproj and overlaps the first attention block
                for nt in range(NQT):
                    proj_chunk(0, wq_sb, qn, 0, nt)
                    proj_chunk(0, wk_sb, kn, 1, nt)
                emit_ss(0)
                bts0 = norm_head(0)
                for nt in range(NQT):
                    proj_chunk(1, wq_sb, qn, 0, nt)
                    proj_chunk(1, wk_sb, kn, 1, nt)
                emit_ss(0)
                norm_tail(0, bts0)
                bts1 = norm_head(1)

                # v projection; vext copies ride the otherwise-idle ACT
                # (same activation table as Square/Sqrt) so the psv slots
                # recycle without touching the DVE queue
                for nm_ in range(NJ):
                    psv = prps.tile([128, HPC * D], F32, name="prv", tag="pr")
                    for c in range(NCT):
                        nc.tensor.matmul(
                            psv[:],
                            lhsT=(
                                x_sb[c][nm_ // 4][
                                    :, (nm_ % 4) * 128 : (nm_ % 4) * 128 + 128
                                ]
                            ),
                            rhs=(wv_sb[c][:]),
                            start=(c == 0),
                            stop=(c == NCT - 1),
                        )
                    nc.scalar.activation(
                        vext[:, nm_, :, 0:D],
                        psv[:].rearrange("p (h d) -> p h d", h=HPC),
                        AF.Copy,
                    )
                norm_tail(1, bts1)

            # ---------------- phase 2: attention + fused out-proj ----------
            with (
                tc.tile_pool(name="simps", bufs=2, space="PSUM") as simps,
                tc.tile_pool(name="ops", bufs=3, space="PSUM") as ops,
                tc.tile_pool(name="ppps", bufs=1, space="PSUM") as ppps,
                tc.tile_pool(name="at", bufs=4) as atp,
                tc.tile_pool(name="nrm", bufs=4) as nrm,
                tc.tile_pool(name="fin", bufs=4) as finp,
            ):
                out_engs = [nc.sync, nc.gpsimd, nc.sync, nc.gpsimd]

                def out_proj_ct(qt, ct):
                    qs_ = slice(qt * 512, (qt + 1) * 512)
                    pp = ppps.tile([128, 512], F32, name="pp", tag="pp")
                    for m in range(2):
                        nc.tensor.matmul(
                            pp[:],
                            lhsT=(wo_sb[m][:, ct * 128 : (ct + 1) * 128]),
                            rhs=(y[m][qt][:]),
                            start=(m == 0),
                            stop=(m == 1),
                        )
                    ot = finp.tile([128, 512], F32, name="ot", tag="ot")
                    nc.vector.tensor_scalar_add(
                        ot[:], pp[:], biasq_sb[:, ct : ct + 1]
                    )
                    out_engs[ct].dma_start(
                        out_d[ct * 128 : (ct + 1) * 128, qs_], ot[:]
                    )

                def av_pair(at_t, j_, oA, oB, hA, hB):
                    nc.tensor.matmul(
                        oA[:],
                        lhsT=(vext[:, j_, hA, :]),
                        rhs=(at_t[:, 0:512]),
                        start=(j_ == 0),
                        stop=(j_ == NJ - 1),
                    )
                    nc.tensor.matmul(
                        oB[:],
                        lhsT=(vext[:, j_, hB, :]),
                        rhs=(at_t[:, 512:1024]),
                        start=(j_ == 0),
                        stop=(j_ == NJ - 1),
                    )

                def normalize(oA, oB, m, qt):
                    # evacuate the accumulators PSUM->SBUF first: the PSUM
                    # banks recycle into the next block without waiting on
                    # the softmax divide chain.  The denominator row gets its
                    # own partition-0 tile (partition_broadcast broadcasts
                    # partition 0).
                    parts = []
                    for o_ps in (oA, oB):
                        oc = nrm.tile([D, 512], F32, name="oc", tag="oc")
                        nc.vector.tensor_copy(oc[:], o_ps[0:D, :])
                        rsb = nrm.tile([1, 512], F32, name="rsb", tag="rsb")
                        nc.vector.tensor_copy(rsb[:], o_ps[D : D + 1, :])
                        parts.append((oc, rsb))
                    for (oc, rsb), base in ((parts[0], 0), (parts[1], 64)):
                        br = nrm.tile([64, 512], F32, name="br", tag="br")
                        nc.gpsimd.partition_broadcast(br[:], rsb[:], channels=64)
                        bri = nrm.tile([64, 512], F32, name="bri", tag="bri")
                        nc.vector.reciprocal_approx_fast(bri[:], br[:])
                        nc.vector.tensor_tensor(
                            y[m][qt][base : base + 64, :],
                            oc[:],
                            bri[:],
                            OP.mult,
                        )

                # the av pair runs one j-step behind the sim pair, ACROSS
                # (m, qt) block boundaries, so the PE's in-order queue never
                # blocks on the current j's exp and blocks hand off
                # seamlessly.  Out-proj column chunks for a finished qt are
                # staggered one per 8 j-steps into the following blocks.
                pend_av = [None]
                pend_pp = []

                def flush_pend():
                    if pend_av[0] is None:
                        return
                    at_t, j_, oA, oB, m_, qt_ = pend_av[0]
                    pend_av[0] = None
                    av_pair(at_t, j_, oA, oB, 2 * m_, 2 * m_ + 1)
                    if j_ == NJ - 1:
                        normalize(oA, oB, m_, qt_)
                        if m_ == 1:
                            pend_pp.extend((qt_, ct) for ct in range(NCT))

                for qt in range(NQT):
                    for m in range(2):
                        qs_ = slice(qt * 512, (qt + 1) * 512)
                        oA = ops.tile([D + 1, 512], F32, name="oA", tag="o")
                        oB = ops.tile([D + 1, 512], F32, name="oB", tag="o")
                        for j in range(NJ):
                            js = slice(j * 128, (j + 1) * 128)
                            sim = simps.tile([128, 1024], F32, name="sim", tag="sim")
                            nc.tensor.matmul(
                                sim[:, 0:512],
                                lhsT=(kn[m][0:64, js]),
                                rhs=(qn[m][0:64, qs_]),
                                start=True,
                                stop=True,
                            )
                            nc.tensor.matmul(
                                sim[:, 512:1024],
                                lhsT=(kn[m][64:128, js]),
                                rhs=(qn[m][64:128, qs_]),
                                start=True,
                                stop=True,
                            )
                            flush_pend()
                            if pend_pp and j % 8 == 2:
                                out_proj_ct(*pend_pp.pop(0))
                            at = atp.tile([128, 1024], BF16, name="at", tag="at")
                            nc.scalar.activation(at[:], sim[:], AF.Exp)
                            pend_av[0] = (at, j, oA, oB, m, qt)
                        # fall through: av(15) + normalize emitted after the
                        # next block's first sim pair
                flush_pend()
                while pend_pp:
                    out_proj_ct(*pend_pp.pop(0))

    nc.finalize()
    return nc


def kernel(x, Wqkv, q_scale, k_scale, Wout, bout):
    global _CACHED_NC, LAST_RESULTS
    x = np.asarray(x, dtype=np.float32)
    Wqkv = np.asarray(Wqkv, dtype=np.float32)
    q_scale = np.asarray(q_scale, dtype=np.float32)
    k_scale = np.asarray(k_scale, dtype=np.float32)
    Wout = np.asarray(Wout, dtype=np.float32)
    bout = np.asarray(bout, dtype=np.float32)

    if _CACHED_NC is None:
        _CACHED_NC = build_nc()
    nc = _CACHED_NC

    H_DIM = HEADS * D
    qsks8 = np.tile((SCALE * q_scale * k_scale).astype(np.float32), 2)[:, None]
    qsks8 = np.ascontiguousarray(qsks8)
    biasq = np.ascontiguousarray((bout / 4.0).astype(np.float32)[:, None])
    onesw = np.zeros((128, 33), dtype=np.float32)
    onesw[0:64, 0] = 1.0
    onesw[64:128, 32] = 1.0
    import ml_dtypes

    in_maps = []
    for core in range(NCORES):
        b = core // 4
        h0 = HPC * (core % 4)
        rs = slice(h0 * D, h0 * D + HPC * D)
        wq = Wqkv[0:H_DIM][rs]
        wk = Wqkv[H_DIM : 2 * H_DIM][rs]
        wv = Wqkv[2 * H_DIM : 3 * H_DIM][rs]
        in_maps.append(
            {
                "x": np.ascontiguousarray(x[b]),
                "wqT": np.ascontiguousarray(wq.T),
                "wkT": np.ascontiguousarray(wk.T),
                "wvT": np.ascontiguousarray(wv.T),
                "woT": np.ascontiguousarray(Wout[:, rs].T).astype(ml_dtypes.bfloat16),
                "qsks8": qsks8,
                "onesw": onesw,
                "biasq": biasq,
            }
        )

    res = run_bass_kernel_spmd(
        nc,
        in_maps,
        core_ids=list(range(NCORES)),
        trace=bool(os.environ.get("BASS_TRACE")),
        **EXTRA_RUN_KWARGS,
    )
    LAST_RESULTS = res

    outs = [np.asarray(res.results[i]["out"], dtype=np.float32) for i in range(NCORES)]
    full = np.empty((B, C, N), dtype=np.float32)
    full[0] = outs[0] + outs[1] + outs[2] + outs[3]
    full[1] = outs[4] + outs[5] + outs[6] + outs[7]
    return full


# revision 41
# speedup vs baseline: 1.1966x; 1.0005x over previous
"""Trainium2 Bass kernel for cosine-similarity ("sparse") attention.

Reference computation (B=2, C=512, N=2048, H=16, D=64, SCALE=8):
    qkv = Wqkv @ x                          # 1x1 conv
    q,k,v -> [B,H,D,N]
    q = l2norm(q, over D) * q_scale ; k = l2norm(k, over D) * k_scale
    sim = (q^T k) * 8 ; attn = softmax(sim, over keys)
    out = Wout @ (attn @ v) + bout

Sharding: 32 (batch, head) pairs across 8 cores -> each core owns one batch
(b = core//4) and 4 heads (h0 = 4*(core%4)).  Each core projects q/k/v for
its heads, runs attention, and computes a partial output projection
Wout[:, its-heads] @ y + bout/4.  Host sums the 4 partials per batch.

Device-side schedule (per core), tuned against NTFF traces:
  - Phase 2 is ACT(exp)-bound at ~1.1us per j-step (128 steps of
    [128,1024] Exp).  Everything else is arranged to hide inside that:
    the av pair runs one j-step behind the sim pair ACROSS block
    boundaries; out-proj column chunks are staggered one per 8 j-steps;
    the o-accumulators are evacuated PSUM->SBUF immediately so their
    banks recycle without waiting on the softmax-denominator divide.
  - qn/kn/y/Wout are bf16 (halves sim LDWEIGHTS + out-proj cost;
    ~0.5-1% rms error, well under the 2e-2 gate).  Norms/sumsq/exp
    stay fp32.
  - Softmax max-subtraction is skipped: sim = 8*cosine is in [-8, 8].
  - l2norm: ones-indicator matmul for sumsq, fused PSUM->SBUF Sqrt on
    ACT, DVE reciprocal; the m=0 chain is emitted before the m=1
    projections so attention can start right after the v projection,
    and the m=1 chain overlaps the first attention block.
  - Startup: small consts + x ride sync/scalar/gpsimd trigger queues
    first; wo comes after the x triggers; the softmax ones-column is a
    GpSimd memset (a scatter DMA here would flood the queues that are
    busy fetching the instruction stream at kernel start).
"""

import os
import sys

import numpy as np

sys.path.insert(0, "/opt/trn_rl_repo")

import concourse.bass as bass  # noqa: E402
import concourse.mybir as mybir  # noqa: E402
from concourse import bacc, tile  # noqa: E402
from concourse.bass_utils import run_bass_kernel_spmd  # noqa: E402

F32 = mybir.dt.float32
F32R = mybir.dt.float32r
BF16 = mybir.dt.bfloat16
AF = mybir.ActivationFunctionType
OP = mybir.AluOpType

B, C, N = 2, 512, 2048
HEADS, D = 16, 64
SCALE = 8.0
NCORES = 8
HPC = 4  # heads per core

_CACHED_NC = None
LAST_RESULTS = None
EXTRA_RUN_KWARGS = {}


def build_nc():
    nc = bacc.Bacc(None, target_bir_lowering=False)

    x_d = nc.declare_dram_parameter("x", [C, N], F32R, isOutput=False)
    wqT_d = nc.declare_dram_parameter("wqT", [C, HPC * D], F32R, isOutput=False)
    wkT_d = nc.declare_dram_parameter("wkT", [C, HPC * D], F32R, isOutput=False)
    wvT_d = nc.declare_dram_parameter("wvT", [C, HPC * D], F32R, isOutput=False)
    woT_d = nc.declare_dram_parameter("woT", [HPC * D, C], BF16, isOutput=False)
    qsks8_d = nc.declare_dram_parameter("qsks8", [128, 1], F32, isOutput=False)
    onesw_d = nc.declare_dram_parameter("onesw", [128, 33], F32R, isOutput=False)
    biasq_d = nc.declare_dram_parameter("biasq", [C, 1], F32, isOutput=False)
    out_d = nc.declare_dram_parameter("out", [C, N], F32, isOutput=True)

    NQT = N // 512  # 4 query chunks of 512
    NJ = N // 128  # 16 key chunks of 128
    NCT = C // 128  # 4 channel chunks of 128

    with tile.TileContext(nc) as tc:
        with (
            tc.tile_pool(name="const", bufs=1) as const,
            tc.tile_pool(name="persist", bufs=1) as persist,
            tc.tile_pool(name="dramp", bufs=1, space="DRAM") as dramp,
        ):
            qsks8_sb = const.tile([128, 1], F32, name="qsks8", tag="qsks8")
            nc.sync.dma_start(qsks8_sb[:], qsks8_d[:])
            biasq_sb = const.tile([128, NCT], F32, name="biasq", tag="biasq")
            nc.sync.dma_start(
                biasq_sb[:], biasq_d[:].rearrange("(c p) o -> p (c o)", p=128)
            )
            # indicator weights: col 0 sums partitions 0-63 (head A), col 32
            # sums partitions 64-127 (head B); middle cols write zeros so the
            # [33, 512] sumsq psum rows land 32-aligned.  (DMA'd from host:
            # engines cannot memset float32r tiles.)
            ones_w = const.tile([128, 33], F32R, name="ones_w", tag="ones_w")
            nc.sync.dma_start(ones_w[:], onesw_d[:])
            wo_sb = [
                const.tile([128, C], BF16, name=f"wo{m}", tag=f"wo{m}")
                for m in range(2)
            ]

            # persistent tensors
            qn = [persist.tile([128, N], BF16, name=f"qn{m}", tag=f"qn{m}") for m in range(2)]
            kn = [persist.tile([128, N], BF16, name=f"kn{m}", tag=f"kn{m}") for m in range(2)]
            y = [
                [
                    persist.tile([128, 512], BF16, name=f"y{m}_{qt}", tag=f"y{m}_{qt}")
                    for qt in range(4)
                ]
                for m in range(2)
            ]
            vext = persist.tile([128, NJ, HPC, D + 1], BF16, name="vext", tag="vext")
            inv_dram = dramp.tile([8, N], F32, name="inv_dram", tag="inv_dram")
            # softmax-denominator ones column: engine memset (bf16) instead
            # of a 2-byte-granular scatter DMA
            nc.gpsimd.memset(vext[:, :, :, D : D + 1], 1.0)

            # ---------------- phase 1: projections + norms ----------------
            with (
                tc.tile_pool(name="xw", bufs=1) as xw,
                tc.tile_pool(name="raw", bufs=1) as rawp,
                tc.tile_pool(name="sq", bufs=3) as sqp,
                tc.tile_pool(name="bb", bufs=4) as bbp,
                tc.tile_pool(name="prps", bufs=4, space="PSUM") as prps,
                tc.tile_pool(name="ssps", bufs=2, space="PSUM") as ssps,
            ):
                # sqrt(sumsq) tiles; head A at row 0 / head B at row 32,
                # filled chunk-by-chunk by fused PSUM->SBUF Sqrt on ACT,
                # then inverted whole by one DVE reciprocal per (t, m)
                srt_tm = [
                    [
                        rawp.tile([33, N], F32, name=f"srt{t}{m}", tag=f"srt{t}{m}")
                        for m in range(2)
                    ]
                    for t in range(2)
                ]
                inv_tm = [
                    [
                        rawp.tile([33, N], F32, name=f"inv{t}{m}", tag=f"inv{t}{m}")
                        for m in range(2)
                    ]
                    for t in range(2)
                ]
                # DMA order tuned for earliest first matmul: wq, x[nt0], wk,
                # x[nt1..3], wv, wo; triggers spread over the trigger queues
                wq_all = xw.tile([128, NCT, HPC * D], F32R, name="wq_all", tag="wq_all")
                nc.scalar.dma_start(
                    wq_all[:], wqT_d[:].rearrange("(c p) d -> p c d", p=128)
                )
                wq_sb = [wq_all[:, c, :] for c in range(NCT)]
                dma_engs = [nc.sync, nc.scalar, nc.gpsimd, nc.sync]
                x_sb = [[None] * NQT for _ in range(NCT)]
                for c in range(NCT):
                    t = xw.tile([128, 512], F32R, name=f"x{c}_0", tag=f"x{c}_0")
                    dma_engs[c].dma_start(t[:], x_d[c * 128 : (c + 1) * 128, 0:512])
                    x_sb[c][0] = t
                wk_all = xw.tile([128, NCT, HPC * D], F32R, name="wk_all", tag="wk_all")
                nc.gpsimd.dma_start(
                    wk_all[:], wkT_d[:].rearrange("(c p) d -> p c d", p=128)
                )
                wk_sb = [wk_all[:, c, :] for c in range(NCT)]
                wv_all = xw.tile([128, NCT, HPC * D], F32R, name="wv_all", tag="wv_all")
                nc.scalar.dma_start(
                    wv_all[:], wvT_d[:].rearrange("(c p) d -> p c d", p=128)
                )
                wv_sb = [wv_all[:, c, :] for c in range(NCT)]
                for nt in range(1, NQT):
                    for c in range(NCT):
                        t = xw.tile([128, 512], F32R, name=f"x{c}_{nt}", tag=f"x{c}_{nt}")
                        dma_engs[c].dma_start(
                            t[:], x_d[c * 128 : (c + 1) * 128, nt * 512 : (nt + 1) * 512]
                        )
                        x_sb[c][nt] = t
                for m in range(2):
                    nc.gpsimd.dma_start(
                        wo_sb[m][:], woT_d[m * 128 : (m + 1) * 128, :]
                    )

                # sumsq matmuls are emitted one proj-chunk late so the PE
                # never waits on the ACT square of the current chunk
                pend_ss = []

                def emit_ss(limit):
                    while len(pend_ss) > limit:
                        sq_t, ti_, m_, nt_ = pend_ss.pop(0)
                        ss = ssps.tile([33, 512], F32, name="ss", tag="ss")
                        nc.tensor.matmul(
                            ss[:], lhsT=(ones_w[:]), rhs=(sq_t[:]), start=True, stop=True
                        )
                        nc.scalar.activation(
                            srt_tm[ti_][m_][:, nt_ * 512 : (nt_ + 1) * 512],
                            ss[:],
                            AF.Sqrt,
                        )

                def proj_chunk(m, w_sb, raws, ti, nt):
                    ps = prps.tile([128, 512], F32, name="pr", tag="pr")
                    for c in range(NCT):
                        nc.tensor.matmul(
                            ps[:],
                            lhsT=(w_sb[c][:, m * 128 : (m + 1) * 128]),
                            rhs=(x_sb[c][nt][:]),
                            start=(c == 0),
                            stop=(c == NCT - 1),
                        )
                    emit_ss(1)
                    nc.vector.tensor_copy(
                        raws[m][:, nt * 512 : (nt + 1) * 512], ps[:]
                    )
                    sq = sqp.tile([128, 512], F32R, name="sq", tag="sq")
                    nc.scalar.activation(sq[:], ps[:], AF.Square)
                    pend_ss.append((sq, ti, m, nt))

                def norm_head(m):
                    # reciprocals + the inverse-norm row DMA roundtrip with
                    # the 64-partition broadcast.  No DVE op here waits on a
                    # DMA, so the DVE queue never blocks behind this.
                    bts = []
                    for ti in range(2):
                        nc.vector.reciprocal_approx_fast(
                            inv_tm[ti][m][:], srt_tm[ti][m][:]
                        )
                        nc.sync.dma_start(
                            inv_dram[4 * ti + 2 * m : 4 * ti + 2 * m + 2, :],
                            inv_tm[ti][m][0:33:32, :],
                        )
                    bt_engs = [nc.sync, nc.gpsimd]
                    for ti in range(2):
                        rowA = 4 * ti + 2 * m
                        bt = bbp.tile([128, N], F32, name="bt", tag="bt")
                        eng = bt_engs[ti]
                        eng.dma_start(
                            bt[0:64, :].unsqueeze(1),
                            inv_dram[rowA : rowA + 1, :].partition_broadcast(64),
                        )
                        eng.dma_start(
                            bt[64:128, :].unsqueeze(1),
                            inv_dram[rowA + 1 : rowA + 2, :].partition_broadcast(64),
                        )
                        bts.append(bt)
                    return bts

                def norm_tail(m, bts):
                    # the actual qn/kn scaling; emitted a projection batch
                    # later so the broadcast DMA has landed by the time the
                    # DVE/GpSimd queues reach these
                    nc.vector.scalar_tensor_tensor(
                        qn[m][:], qn[m][:], qsks8_sb[:], bts[0][:],
                        OP.mult, OP.mult,
                    )
                    nc.gpsimd.tensor_tensor(
                        kn[m][:], kn[m][:], bts[1][:], OP.mult
                    )

                # m=0 projections, then its norm chain split around the m=1
                # projections (the DMA roundtrip overlaps them); m=1's chain
                # splits around vproj and overlaps the first attention block
                for nt in range(NQT):
                    proj_chunk(0, wq_sb, qn, 0, nt)
                    proj_chunk(0, wk_sb, kn, 1, nt)
                emit_ss(0)
                bts0 = norm_head(0)
                for nt in range(NQT):
                    proj_chunk(1, wq_sb, qn, 0, nt)
                    proj_chunk(1, wk_sb, kn, 1, nt)
                emit_ss(0)
                norm_tail(0, bts0)
                bts1 = norm_head(1)

                # v projection; vext copies ride the otherwise-idle ACT
                # (same activation table as Square/Sqrt) so the psv slots
                # recycle without touching the DVE queue
                for nm_ in range(NJ):
                    psv = prps.tile([128, HPC * D], F32, name="prv", tag="pr")
                    for c in range(NCT):
                        nc.tensor.matmul(
                            psv[:],
                            lhsT=(
                                x_sb[c][nm_ // 4][
                                    :, (nm_ % 4) * 128 : (nm_ % 4) * 128 + 128
                                ]
                            ),
                            rhs=(wv_sb[c][:]),
                            start=(c == 0),
                            stop=(c == NCT - 1),
                        )
                    nc.scalar.activation(
                        vext[:, nm_, :, 0:D],
                        psv[:].rearrange("p (h d) -> p h d", h=HPC),
                        AF.Copy,
                    )
                norm_tail(1, bts1)

            # ---------------- phase 2: attention + fused out-proj ----------
            with (
                tc.tile_pool(name="simps", bufs=2, space="PSUM") as simps,
                tc.tile_pool(name="ops", bufs=3, space="PSUM") as ops,
                tc.tile_pool(name="ppps", bufs=1, space="PSUM") as ppps,
                tc.tile_pool(name="at", bufs=4) as atp,
                tc.tile_pool(name="nrm", bufs=4) as nrm,
                tc.tile_pool(name="fin", bufs=4) as finp,
            ):
                out_engs = [nc.sync, nc.gpsimd, nc.sync, nc.gpsimd]

                def out_proj_halves(qt, ct):
                    # the two accumulation halves of one out-proj chunk are
                    # emitted at separate stagger points (PSUM accumulation
                    # groups may interleave with other matmuls), so each PE
                    # insertion stays within the per-j ACT slack
                    qs_ = slice(qt * 512, (qt + 1) * 512)
                    box = {}

                    def start_half():
                        pp = ppps.tile([128, 512], F32, name="pp", tag="pp")
                        box["pp"] = pp
                        nc.tensor.matmul(
                            pp[:],
                            lhsT=(wo_sb[0][:, ct * 128 : (ct + 1) * 128]),
                            rhs=(y[0][qt][:]),
                            start=True,
                            stop=False,
                        )

                    def finish_half():
                        pp = box["pp"]
                        nc.tensor.matmul(
                            pp[:],
                            lhsT=(wo_sb[1][:, ct * 128 : (ct + 1) * 128]),
                            rhs=(y[1][qt][:]),
                            start=False,
                            stop=True,
                        )
                        ot = finp.tile([128, 512], F32, name="ot", tag="ot")
                        nc.vector.tensor_scalar_add(
                            ot[:], pp[:], biasq_sb[:, ct : ct + 1]
                        )
                        out_engs[ct].dma_start(
                            out_d[ct * 128 : (ct + 1) * 128, qs_], ot[:]
                        )

                    return start_half, finish_half

                def av_pair(at_t, j_, oA, oB, hA, hB):
                    nc.tensor.matmul(
                        oA[:],
                        lhsT=(vext[:, j_, hA, :]),
                        rhs=(at_t[:, 0:512]),
                        start=(j_ == 0),
                        stop=(j_ == NJ - 1),
                    )
                    nc.tensor.matmul(
                        oB[:],
                        lhsT=(vext[:, j_, hB, :]),
                        rhs=(at_t[:, 512:1024]),
                        start=(j_ == 0),
                        stop=(j_ == NJ - 1),
                    )

                def normalize(oA, oB, m, qt):
                    # evacuate the accumulators PSUM->SBUF first: the PSUM
                    # banks recycle into the next block without waiting on
                    # the softmax divide chain.  The denominator row gets its
                    # own partition-0 tile (partition_broadcast broadcasts
                    # partition 0).
                    parts = []
                    for o_ps in (oA, oB):
                        oc = nrm.tile([D, 512], F32, name="oc", tag="oc")
                        nc.vector.tensor_copy(oc[:], o_ps[0:D, :])
                        rsb = nrm.tile([1, 512], F32, name="rsb", tag="rsb")
                        nc.vector.tensor_copy(rsb[:], o_ps[D : D + 1, :])
                        parts.append((oc, rsb))
                    for (oc, rsb), base in ((parts[0], 0), (parts[1], 64)):
                        br = nrm.tile([64, 512], F32, name="br", tag="br")
                        nc.gpsimd.partition_broadcast(br[:], rsb[:], channels=64)
                        bri = nrm.tile([64, 512], F32, name="bri", tag="bri")
                        nc.vector.reciprocal_approx_fast(bri[:], br[:])
                        nc.vector.tensor_tensor(
                            y[m][qt][base : base + 64, :],
                            oc[:],
                            bri[:],
                            OP.mult,
                        )

                # the av pair runs one j-step behind the sim pair, ACROSS
                # (m, qt) block boundaries, so the PE's in-order queue never
                # blocks on the current j's exp and blocks hand off
                # seamlessly.  Out-proj column chunks for a finished qt are
                # staggered one per 8 j-steps into the following blocks.
                pend_av = [None]
                pend_pp = []

                def flush_pend():
                    if pend_av[0] is None:
                        return
                    at_t, j_, oA, oB, m_, qt_ = pend_av[0]
                    pend_av[0] = None
                    av_pair(at_t, j_, oA, oB, 2 * m_, 2 * m_ + 1)
                    if j_ == NJ - 1:
                        normalize(oA, oB, m_, qt_)
                        if m_ == 1:
                            for ct in range(NCT):
                                pend_pp.extend(out_proj_halves(qt_, ct))

                for qt in range(NQT):
                    for m in range(2):
                        qs_ = slice(qt * 512, (qt + 1) * 512)
                        oA = ops.tile([D + 1, 512], F32, name="oA", tag="o")
                        oB = ops.tile([D + 1, 512], F32, name="oB", tag="o")
                        for j in range(NJ):
                            js = slice(j * 128, (j + 1) * 128)
                            sim = simps.tile([128, 1024], F32, name="sim", tag="sim")
                            nc.tensor.matmul(
                                sim[:, 0:512],
                                lhsT=(kn[m][0:64, js]),
                                rhs=(qn[m][0:64, qs_]),
                                start=True,
                                stop=True,
                            )
                            nc.tensor.matmul(
                                sim[:, 512:1024],
                                lhsT=(kn[m][64:128, js]),
                                rhs=(qn[m][64:128, qs_]),
                                start=True,
                                stop=True,
                            )
                            flush_pend()
                            if pend_pp and j % 4 == 3:
                                pend_pp.pop(0)()
                            at = atp.tile([128, 1024], BF16, name="at", tag="at")
                            nc.scalar.activation(at[:], sim[:], AF.Exp)
                            pend_av[0] = (at, j, oA, oB, m, qt)
                        # fall through: av(15) + normalize emitted after the
                        # next block's first sim pair
                flush_pend()
                while pend_pp:
                    pend_pp.pop(0)()

    nc.finalize()
    return nc


def kernel(x, Wqkv, q_scale, k_scale, Wout, bout):
    global _CACHED_NC, LAST_RESULTS
    x = np.asarray(x, dtype=np.float32)
    Wqkv = np.asarray(Wqkv, dtype=np.float32)
    q_scale = np.asarray(q_scale, dtype=np.float32)
    k_scale = np.asarray(k_scale, dtype=np.float32)
    Wout = np.asarray(Wout, dtype=np.float32)
    bout = np.asarray(bout, dtype=np.float32)

    if _CACHED_NC is None:
        _CACHED_NC = build_nc()
    nc = _CACHED_NC

    H_DIM = HEADS * D
    qsks8 = np.tile((SCALE * q_scale * k_scale).astype(np.float32), 2)[:, None]
    qsks8 = np.ascontiguousarray(qsks8)
    biasq = np.ascontiguousarray((bout / 4.0).astype(np.float32)[:, None])
    onesw = np.zeros((128, 33), dtype=np.float32)
    onesw[0:64, 0] = 1.0
    onesw[64:128, 32] = 1.0
    import ml_dtypes

    in_maps = []
    for core in range(NCORES):
        b = core // 4
        h0 = HPC * (core % 4)
        rs = slice(h0 * D, h0 * D + HPC * D)
        wq = Wqkv[0:H_DIM][rs]
        wk = Wqkv[H_DIM : 2 * H_DIM][rs]
        wv = Wqkv[2 * H_DIM : 3 * H_DIM][rs]
        in_maps.append(
            {
                "x": np.ascontiguousarray(x[b]),
                "wqT": np.ascontiguousarray(wq.T),
                "wkT": np.ascontiguousarray(wk.T),
                "wvT": np.ascontiguousarray(wv.T),
                "woT": np.ascontiguousarray(Wout[:, rs].T).astype(ml_dtypes.bfloat16),
                "qsks8": qsks8,
                "onesw": onesw,
                "biasq": biasq,
            }
        )

    res = run_bass_kernel_spmd(
        nc,
        in_maps,
        core_ids=list(range(NCORES)),
        trace=bool(os.environ.get("BASS_TRACE")),
        **EXTRA_RUN_KWARGS,
    )
    LAST_RESULTS = res

    outs = [np.asarray(res.results[i]["out"], dtype=np.float32) for i in range(NCORES)]
    full = np.empty((B, C, N), dtype=np.float32)
    full[0] = outs[0] + outs[1] + outs[2] + outs[3]
    full[1] = outs[4] + outs[5] + outs[6] + outs[7]
    return full
